# revision 12
# baseline (speedup 1.0000x reference)
"""Trainium2 Bass kernel for nn_MultiHeadAttention_72765335929540.

Reference semantics (B=8, S=2048, D=512, H=8 identical heads, d_k=d_v=64):
    q = query @ Wq + bq;  k = key @ Wk + bk;  v = key @ Wv + bv   (bug: v from key)
    scores = q k^T / 8 (+ causal mask if training);  att = softmax(scores)
    head = att @ v;  out = tile(head, 8) @ Wo + bo = head @ Wo_eff + bo
where Wo_eff = sum_h Wo[64h:64h+64].  `value` is never read.

Distribution: data-parallel, one batch element per NeuronCore (8 cores).

v4 design:
  * Inputs stream with 4KB-contiguous HBM reads: partition p of a 256-row
    half-group tile holds rows 2p, 2p+1 (cast-load f32->bf16).  This is a
    stride-2 token permutation tau = 128*(2h+t)+i <-> token 256h+2i+t, fixed
    up in the causal-mask constants and the strided output scatter.  All 16
    input DMAs (plus consts) are issued up-front on gpsimd (SWDGE) so the
    queue streams continuously from t~0.
  * X^T via PE transposes of [128,128] sub-blocks into one [128,1024] bf16
    PSUM bank per (tensor, half-group), evicted in a single DVE copy.
  * Causal mask for the permuted order: two 0/1 [128,256] patterns
    MULTIPLIED into exp(scores) on the gpsimd engine (PE does no mask work,
    scores never need the -1e30 add).
  * Output written as bf16 (rel-err budget is 2e-2), halving write traffic;
    scattered back to natural token order by a strided DMA.
  * Engine balance: exp + finalize scaling on ACT; transpose/projection/
    v'/head evictions + reciprocal on DVE; mask multiplies + all loads on
    gpsimd; output stores on sync (HWDGE).

Per-piece sweep p=0..3 (512 permuted queries) over key classes C=0..4p+3
(128 permuted keys): scoresT -> exp -> mask -> headT' accumulation with a
ones-column in v' providing the softmax denominator; normalization fused
into the output projection eviction.
"""
import sys

sys.path.insert(0, "/opt/trn_rl_repo")

import numpy as np
import ml_dtypes

import concourse.bass as bass
import concourse.mybir as mybir
import concourse.tile as tile
from concourse.bass_utils import run_bass_kernel_spmd

BF = mybir.dt.bfloat16
F32 = mybir.dt.float32
F8 = mybir.dt.float8e4
S, D, DK = 2048, 512, 64
NCLS = 16              # 16 classes of 128 permuted tokens
NHG = 8                # 8 half-groups of 256 token rows
H = 8

# ---------------------------------------------------------------------------
# walrus workaround: this build's ISA structs hold few semaphore waits per
# instruction; split the excess onto same-engine NoOps (1 wait each).
_ws_counter = [0]
_CTRL_TYPES = ("InstDrain", "InstNoOp", "InstEventSemaphore", "InstBranch")


def _split_sync_waits(nc, max_waits=1, max_updates=2):
    for f in nc.m.functions:
        for blk in f.blocks:
            insts = blk.instructions
            i = 0
            while i < len(insts):
                inst = insts[i]
                si = inst.sync_info
                if si is None:
                    i += 1
                    continue
                ctrl = type(inst).__name__ in _CTRL_TYPES
                max_w = 1 if ctrl else max_waits
                max_u = 1 if ctrl else max_updates
                waits = list(si.on_wait)
                updates = list(si.on_update)
                if len(waits) <= max_w and len(updates) <= max_u:
                    i += 1
                    continue
                keep_w = waits[-max_w:] if len(waits) > max_w else waits
                extra_w = waits[:-max_w] if len(waits) > max_w else []
                keep_u = updates[:max_u] if len(updates) > max_u else updates
                extra_u = updates[max_u:] if len(updates) > max_u else []
                inst.sync_info = mybir.SyncInfo(on_wait=keep_w, on_update=keep_u)
                pre, post = [], []
                for w in extra_w:
                    _ws_counter[0] += 1
                    nop = mybir.InstNoOp(name=f"WSPLIT-{_ws_counter[0]}", ins=[], outs=[])
                    nop.engine = inst.engine
                    nop.sync_info = mybir.SyncInfo(on_wait=[w], on_update=[])
                    pre.append(nop)
                for u in extra_u:
                    _ws_counter[0] += 1
                    nop = mybir.InstNoOp(name=f"USPLIT-{_ws_counter[0]}", ins=[], outs=[])
                    nop.engine = inst.engine
                    nop.sync_info = mybir.SyncInfo(on_wait=[], on_update=[u])
                    post.append(nop)
                for k, nop in enumerate(pre):
                    insts.insert(i + k, nop)
                for k, nop in enumerate(post):
                    insts.insert(i + len(pre) + 1 + k, nop)
                i += len(pre) + 1 + len(post)


# ---------------------------------------------------------------------------
def _build_nc(masked: bool):
    nc = bass.Bass()
    q_d = nc.declare_dram_parameter("query", [S, D], BF, isOutput=False)
    k_d = nc.declare_dram_parameter("key", [S, D], BF, isOutput=False)
    # packed constants: one bf16 [128, CW] blob + one f32 [128, 2] blob
    # layout (cols): wq 256 | wkv 512 | frhs 512 | maska 256 | maskb 256 | id 128
    CW = 256 + 512 + 512 + 256 + 256 + 128
    cb_d = nc.declare_dram_parameter("cb", [128, CW], BF, isOutput=False)
    cf_d = nc.declare_dram_parameter("cf", [128, 2], F32, isOutput=False)
    out_d = nc.declare_dram_parameter("out", [S, D], BF, isOutput=True)
    warm_d = nc.declare_dram_parameter("warm", [128, 1], F32, isOutput=True)

    Exp = mybir.ActivationFunctionType.Exp

    with tile.TileContext(nc) as tc:
        with (
            tc.tile_pool(name="pers", bufs=1) as pers,
            tc.tile_pool(name="hts", bufs=2) as hts,
            tc.tile_pool(name="osb", bufs=3) as osb,
            tc.tile_pool(name="ps", bufs=1, space="PSUM") as ps,
        ):
            # ---- const loads: two packed DMAs (gpsimd SWDGE) ---------------
            cb_sb = pers.tile([128, CW], BF, tag="cb")
            nc.gpsimd.dma_start(cb_sb[:], cb_d[:])
            cf_sb = pers.tile([128, 2], F32, tag="cf")
            nc.gpsimd.dma_start(cf_sb[:], cf_d[:])
            wq_sb = cb_sb[:, 0:256]
            wkv_sb = cb_sb[:, 256:768]
            frhs_sb = cb_sb[0:DK + 1, 768:1280]
            maska_sb = cb_sb[:, 1280:1536]
            maskb_sb = cb_sb[:, 1536:1792]
            id_sb = cb_sb[:, 1792:1920]
            bq_sb = cf_sb[0:DK, 0:1]
            bkv_sb = cf_sb[:, 1:2]

            # ---- input loads, all issued up-front (gpsimd SWDGE) -----------
            # half-group h tile: [128, 1024] bf16, (p, 512*t + d) =
            # X[256h + 2p + t, d]  -> 4KB-contiguous f32 reads per partition.
            knat = [pers.tile([128, 1024], BF, tag=f"knat{h}", name=f"knat{h}")
                    for h in range(NHG)]
            qnat = [pers.tile([128, 1024], BF, tag=f"qnat{h}", name=f"qnat{h}")
                    for h in range(NHG)]
            # k on the scalar HWDGE queue, q on the sync HWDGE queue: two
            # independent DMA queues, ~600ns issue each, parallel engines.
            for h in range(NHG):
                nc.scalar.dma_start(
                    knat[h][:].rearrange("p (two d) -> p two d", two=2),
                    k_d[256 * h:256 * (h + 1), :].rearrange(
                        "(p two) d -> p two d", p=128))
                nc.sync.dma_start(
                    qnat[h][:].rearrange("p (two d) -> p two d", two=2),
                    q_d[256 * h:256 * (h + 1), :].rearrange(
                        "(p two) d -> p two d", p=128))

            # ---- persistent activations -----------------------------------
            # xT layout: col 1024h + 512t + 128c + i  (c = D-chunk, i =
            # intra-class token index, classes C = 2h+t).
            xqT = pers.tile([128, 8192], BF, tag="xqT")
            xkT = pers.tile([128, 8192], BF, tag="xkT")
            # fp8 score operands: cols [0:S) hold the tensor, [S:2S) are
            # zeros (the DoubleRow j=1 half contributes 0 to the contraction)
            qT8 = pers.tile([DK, 2 * S], F8, tag="qT8")
            kT8 = pers.tile([DK, 2 * S], F8, tag="kT8")
            vT = pers.tile([DK, S], BF, tag="vT")
            nc.gpsimd.memset(qT8[:, S:2 * S], 0.0)
            nc.gpsimd.memset(kT8[:, S:2 * S], 0.0)
            # v' for all 16 classes: class C at cols [65C, 65C+64], ones col
            # at 65C+64 (one big memset; evictions overwrite the value cols).
            vpr = pers.tile([128, 65 * NCLS], BF, tag="vpr")
            nc.gpsimd.memset(vpr[:], 1.0)

            # pT storage per class: W[C] = 2048 - 256*(C>>1)
            Ws = [(S - 256 * (C >> 1)) if masked else S for C in range(NCLS)]
            pts = [pers.tile([128, Ws[C]], BF, tag=f"pt{C}", name=f"pt{C}")
                   for C in range(NCLS)]

            # warm output (declared param must be written)
            wu2 = pers.tile([128, 1], F32, tag="wu2")
            nc.gpsimd.memset(wu2[:], 0.0)
            nc.gpsimd.dma_start(warm_d[:], wu2[:])

            # ---- per-half-group setup -------------------------------------
            # proj rhs view: [p, t, i] for fixed (h, c)
            xqT_p = xqT[:].rearrange("p (h t c i) -> p h t c i",
                                     h=NHG, t=2, c=4, i=128)
            xkT_p = xkT[:].rearrange("p (h t c i) -> p h t c i",
                                     h=NHG, t=2, c=4, i=128)

            def setup_hg(h):
                sl = slice(256 * h, 256 * (h + 1))
                xsl = slice(1024 * h, 1024 * (h + 1))
                # PE transposes into one bf16 psum bank per tensor, single
                # DVE eviction each
                for nat, xT, who in ((knat[h], xkT, "k"), (qnat[h], xqT, "q")):
                    tr = ps.tile([128, 1024], BF, tag="tr",
                                 name=f"tr_{who}{h}", bufs=2)
                    for t in range(2):
                        for c in range(4):
                            o = 512 * t + 128 * c
                            nc.tensor.transpose(tr[:, o:o + 128],
                                                nat[:, o:o + 128], id_sb)
                    nc.vector.tensor_copy(xT[:, xsl], tr[:])
                # kv projection
                pkv = ps.tile([128, 256], F32, tag="pj", name=f"pkv{h}", bufs=1)
                for cc in range(4):
                    nc.tensor.matmul(pkv[:],
                                     lhsT=wkv_sb[:, cc * 128:(cc + 1) * 128],
                                     rhs=xkT_p[:, h, :, cc, :],
                                     start=(cc == 0), stop=(cc == 3))
                nc.vector.tensor_scalar_add(kT8[:, sl], pkv[0:DK, :],
                                            bkv_sb[0:DK, :])
                nc.vector.tensor_scalar_add(vT[:, sl], pkv[DK:128, :],
                                            bkv_sb[DK:128, :])
                # q projection
                pq = ps.tile([DK, 256], F32, tag="pj", name=f"pq{h}", bufs=1)
                for cc in range(4):
                    nc.tensor.matmul(pq[:],
                                     lhsT=wq_sb[:, cc * DK:(cc + 1) * DK],
                                     rhs=xqT_p[:, h, :, cc, :],
                                     start=(cc == 0), stop=(cc == 3))
                nc.vector.tensor_scalar_add(qT8[:, sl], pq[:], bq_sb)
                # v' for the two classes of this half-group (one psum, one
                # strided eviction)
                pv = ps.tile([128, 128], BF, tag="pv", name=f"pv{h}", bufs=1)
                for t in range(2):
                    C = 2 * h + t
                    nc.tensor.transpose(pv[:, 64 * t:64 * t + 64],
                                        vT[:, 128 * C:128 * (C + 1)],
                                        id_sb[0:64, 0:64])
                nc.vector.tensor_copy(
                    vpr[:].rearrange("p (C x) -> p C x", x=65)
                        [:, 2 * h:2 * h + 2, 0:64],
                    pv[:].rearrange("p (C x) -> p C x", x=64))

            # ---- per-piece attention sweep --------------------------------
            out_v = out_d[:].rearrange("(h p two) d -> h two p d", h=NHG, p=128)

            def piece(p):
                Cmax = 4 * p + 3 if masked else NCLS - 1
                # scores + exp + mask
                for C in range(Cmax + 1):
                    h2 = C >> 1
                    if masked and h2 == 2 * p + 1:
                        local0, w = 256, 256
                    else:
                        local0, w = 0, 512
                    g0 = 512 * p + local0
                    psc = ps.tile([128, 512], F32, tag="sc", name=f"sc_{C}_{p}",
                                  bufs=2)
                    k8v = kT8[:].rearrange("p (j t) -> p j t", j=2)
                    q8v = qT8[:].rearrange("p (j t) -> p j t", j=2)
                    nc.tensor.matmul(psc[:, 0:w],
                                     lhsT=k8v[:, :, 128 * C:128 * (C + 1)],
                                     rhs=q8v[:, :, g0:g0 + w],
                                     start=True, stop=True,
                                     perf_mode=mybir.MatmulPerfMode.DoubleRow,
                                     skip_group_check=True)
                    if masked:
                        x = g0 - 512 * (h2 >> 1) - 256 * (h2 & 1)
                    else:
                        x = 512 * p
                    nc.scalar.activation(pts[C][:, x:x + w], psc[:, 0:w],
                                         Exp, scale=0.125)
                    if masked and (h2 == 2 * p or h2 == 2 * p + 1):
                        msk = maska_sb if (C & 1) == 0 else maskb_sb
                        nc.gpsimd.tensor_tensor(pts[C][:, 0:256],
                                                pts[C][:, 0:256], msk,
                                                mybir.AluOpType.mult)
                # head accumulation
                hacc = ps.tile([DK + 1, 512], F32, tag="ha", name=f"ha{p}",
                               bufs=1)
                for C in range(Cmax + 1):
                    h2 = C >> 1
                    if masked and h2 == 2 * p + 1:
                        lo, w = 256, 256
                    else:
                        lo, w = 0, 512
                    if masked:
                        x = 512 * p + lo - 512 * (h2 >> 1) - 256 * (h2 & 1)
                    else:
                        x = 512 * p
                    nc.tensor.matmul(hacc[:, lo:lo + w],
                                     lhsT=vpr[:, 65 * C:65 * (C + 1)],
                                     rhs=pts[C][:, x:x + w],
                                     start=(C == 0), stop=(C == Cmax),
                                     skip_group_check=True)
                ht4 = hts.tile([DK + 1, 512], BF, tag="ht", name=f"ht{p}")
                nc.vector.tensor_copy(ht4[:], hacc[:])
                # finalize the 4 classes of this piece; l-row -> columns via
                # one PE transpose + one batched reciprocal
                # bf16 transpose outs must land 4-byte aligned: stride-2 cols
                pl = ps.tile([128, 8], BF, tag="pv", name=f"pl{p}", bufs=1)
                for b in range(4):
                    nc.tensor.transpose(pl[:, 2 * b:2 * b + 1],
                                        ht4[DK:DK + 1, 128 * b:128 * (b + 1)],
                                        id_sb[64:65, 64:65])
                r = hts.tile([128, 4], F32, tag="r", name=f"r{p}")
                nc.vector.reciprocal(
                    r[:], pl[:].rearrange("p (b two) -> p b two", two=2)[:, :, 0])
                for b in range(4):
                    C = 4 * p + b
                    po = ps.tile([128, 512], F32, tag="po", name=f"po{C}",
                                 bufs=1)
                    nc.tensor.matmul(po[:], lhsT=ht4[:, 128 * b:128 * (b + 1)],
                                     rhs=frhs_sb, start=True, stop=True)
                    ot = osb.tile([128, D], BF, tag="ot", name=f"ot{C}")
                    nc.scalar.mul(ot[:], po[:], r[:, b:b + 1])
                    nc.gpsimd.dma_start(out_v[C >> 1, C & 1], ot[:])

            for h in range(NHG):
                setup_hg(h)
                if h % 2 == 1:
                    piece(h // 2)

    _split_sync_waits(nc)
    return nc


_NC_CACHE = {}


def _get_nc(masked: bool):
    if masked not in _NC_CACHE:
        _NC_CACHE[masked] = _build_nc(masked)
    return _NC_CACHE[masked]


# ---------------------------------------------------------------------------
def kernel(query, key, value, Wq, bq, Wk, bk, Wv, bv, Wo, bo, training):
    query = np.asarray(query, dtype=np.float32)
    key = np.asarray(key, dtype=np.float32)
    Wq = np.asarray(Wq, dtype=np.float64)
    Wk = np.asarray(Wk, dtype=np.float64)
    Wv = np.asarray(Wv, dtype=np.float64)
    Wo = np.asarray(Wo, dtype=np.float64)
    bq_h = np.asarray(bq, dtype=np.float32).reshape(DK, 1)
    bk_h = np.asarray(bk, dtype=np.float32).reshape(DK, 1)
    bv_h = np.asarray(bv, dtype=np.float32).reshape(DK, 1)
    bo_h = np.asarray(bo, dtype=np.float64)
    masked = bool(np.asarray(training).item())

    B = query.shape[0]
    query_bf = query.astype(ml_dtypes.bfloat16)
    key_bf = key.astype(ml_dtypes.bfloat16)
    wq_h = Wq.astype(ml_dtypes.bfloat16)
    wkv_h = np.concatenate([Wk, Wv], axis=1).astype(ml_dtypes.bfloat16)
    bkv_h = np.concatenate([bk_h, bv_h], axis=0)
    wo_eff = Wo.reshape(H, DK, D).sum(axis=0)
    frhs_h = np.concatenate([wo_eff, bo_h[None, :]], axis=0).astype(ml_dtypes.bfloat16)
    jj, ii = np.meshgrid(np.arange(128), np.arange(128), indexing="ij")
    t1v = (jj <= ii).astype(np.float64)   # t2<=t1: key j visible iff j<=i
    t0v = (jj < ii).astype(np.float64)    # t2>t1:  key j visible iff j<i
    maska_h = np.concatenate([t1v, t1v], axis=1).astype(ml_dtypes.bfloat16)
    maskb_h = np.concatenate([t0v, t1v], axis=1).astype(ml_dtypes.bfloat16)
    id_h = np.eye(128, dtype=ml_dtypes.bfloat16)

    # packed const blobs (see _build_nc layout)
    wq_p = np.zeros((128, 256), ml_dtypes.bfloat16)
    for cc in range(4):
        wq_p[:, cc * 64:(cc + 1) * 64] = wq_h[cc * 128:(cc + 1) * 128, :]
    wkv_p = np.zeros((128, 512), ml_dtypes.bfloat16)
    for cc in range(4):
        wkv_p[:, cc * 128:(cc + 1) * 128] = wkv_h[cc * 128:(cc + 1) * 128, :]
    frhs_p = np.zeros((128, 512), ml_dtypes.bfloat16)
    frhs_p[0:DK + 1, :] = frhs_h
    cb = np.concatenate([wq_p, wkv_p, frhs_p, maska_h, maskb_h, id_h], axis=1)
    cf = np.zeros((128, 2), np.float32)
    cf[0:DK, 0] = bq_h[:, 0]
    cf[:, 1] = bkv_h[:, 0]
    consts = {"cb": np.ascontiguousarray(cb), "cf": cf}
    in_maps = [dict(consts, query=np.ascontiguousarray(query_bf[i]),
                    key=np.ascontiguousarray(key_bf[i])) for i in range(B)]

    global _last_in_maps
    _last_in_maps = in_maps
    nc = _get_nc(masked)
    res = run_bass_kernel_spmd(nc, in_maps, core_ids=list(range(B)))
    return np.stack([np.asarray(res.results[i]["out"], dtype=np.float32)
                     for i in range(B)])


# revision 13
# speedup vs baseline: 1.0889x; 1.0889x over previous
"""Trainium2 Bass kernel for nn_MultiHeadAttention_72765335929540.

Reference semantics (B=8, S=2048, D=512, H=8 identical heads, d_k=d_v=64):
    q = query @ Wq + bq;  k = key @ Wk + bk;  v = key @ Wv + bv   (bug: v from key)
    scores = q k^T / 8 (+ causal mask if training);  att = softmax(scores)
    head = att @ v;  out = tile(head, 8) @ Wo + bo = head @ Wo_eff + bo
where Wo_eff = sum_h Wo[64h:64h+64].  `value` is never read.

Distribution: data-parallel, one batch element per NeuronCore (8 cores).

v4 design:
  * Inputs stream with 4KB-contiguous HBM reads: partition p of a 256-row
    half-group tile holds rows 2p, 2p+1 (cast-load f32->bf16).  This is a
    stride-2 token permutation tau = 128*(2h+t)+i <-> token 256h+2i+t, fixed
    up in the causal-mask constants and the strided output scatter.  All 16
    input DMAs (plus consts) are issued up-front on gpsimd (SWDGE) so the
    queue streams continuously from t~0.
  * X^T via PE transposes of [128,128] sub-blocks into one [128,1024] bf16
    PSUM bank per (tensor, half-group), evicted in a single DVE copy.
  * Causal mask for the permuted order: two 0/1 [128,256] patterns
    MULTIPLIED into exp(scores) on the gpsimd engine (PE does no mask work,
    scores never need the -1e30 add).
  * Output written as bf16 (rel-err budget is 2e-2), halving write traffic;
    scattered back to natural token order by a strided DMA.
  * Engine balance: exp + finalize scaling on ACT; transpose/projection/
    v'/head evictions + reciprocal on DVE; mask multiplies + all loads on
    gpsimd; output stores on sync (HWDGE).

Per-piece sweep p=0..3 (512 permuted queries) over key classes C=0..4p+3
(128 permuted keys): scoresT -> exp -> mask -> headT' accumulation with a
ones-column in v' providing the softmax denominator; normalization fused
into the output projection eviction.
"""
import sys

sys.path.insert(0, "/opt/trn_rl_repo")

import numpy as np
import ml_dtypes

import concourse.bass as bass
import concourse.mybir as mybir
import concourse.tile as tile
from concourse.bass_utils import run_bass_kernel_spmd

BF = mybir.dt.bfloat16
F32 = mybir.dt.float32
F8 = mybir.dt.float8e4
S, D, DK = 2048, 512, 64
NCLS = 16              # 16 classes of 128 permuted tokens
NHG = 8                # 8 half-groups of 256 token rows
H = 8

# ---------------------------------------------------------------------------
# walrus workaround: this build's ISA structs hold few semaphore waits per
# instruction; split the excess onto same-engine NoOps (1 wait each).
_ws_counter = [0]
_CTRL_TYPES = ("InstDrain", "InstNoOp", "InstEventSemaphore", "InstBranch")


def _split_sync_waits(nc, max_waits=1, max_updates=2):
    for f in nc.m.functions:
        for blk in f.blocks:
            insts = blk.instructions
            i = 0
            while i < len(insts):
                inst = insts[i]
                si = inst.sync_info
                if si is None:
                    i += 1
                    continue
                ctrl = type(inst).__name__ in _CTRL_TYPES
                max_w = 1 if ctrl else max_waits
                max_u = 1 if ctrl else max_updates
                waits = list(si.on_wait)
                updates = list(si.on_update)
                if len(waits) <= max_w and len(updates) <= max_u:
                    i += 1
                    continue
                keep_w = waits[-max_w:] if len(waits) > max_w else waits
                extra_w = waits[:-max_w] if len(waits) > max_w else []
                keep_u = updates[:max_u] if len(updates) > max_u else updates
                extra_u = updates[max_u:] if len(updates) > max_u else []
                inst.sync_info = mybir.SyncInfo(on_wait=keep_w, on_update=keep_u)
                pre, post = [], []
                for w in extra_w:
                    _ws_counter[0] += 1
                    nop = mybir.InstNoOp(name=f"WSPLIT-{_ws_counter[0]}", ins=[], outs=[])
                    nop.engine = inst.engine
                    nop.sync_info = mybir.SyncInfo(on_wait=[w], on_update=[])
                    pre.append(nop)
                for u in extra_u:
                    _ws_counter[0] += 1
                    nop = mybir.InstNoOp(name=f"USPLIT-{_ws_counter[0]}", ins=[], outs=[])
                    nop.engine = inst.engine
                    nop.sync_info = mybir.SyncInfo(on_wait=[], on_update=[u])
                    post.append(nop)
                for k, nop in enumerate(pre):
                    insts.insert(i + k, nop)
                for k, nop in enumerate(post):
                    insts.insert(i + len(pre) + 1 + k, nop)
                i += len(pre) + 1 + len(post)


# ---------------------------------------------------------------------------
def _build_nc(masked: bool):
    nc = bass.Bass()
    q_d = nc.declare_dram_parameter("query", [S, D], BF, isOutput=False)
    k_d = nc.declare_dram_parameter("key", [S, D], BF, isOutput=False)
    # packed constants: one bf16 [128, CW] blob + one f32 [128, 2] blob
    # layout (cols): wq 256 | wkv 512 | frhs 512 | maska 256 | maskb 256 | id 128
    CW = 256 + 512 + 512 + 256 + 256 + 128
    cb_d = nc.declare_dram_parameter("cb", [128, CW], BF, isOutput=False)
    cf_d = nc.declare_dram_parameter("cf", [128, 2], F32, isOutput=False)
    out_d = nc.declare_dram_parameter("out", [S, D], BF, isOutput=True)
    warm_d = nc.declare_dram_parameter("warm", [128, 1], F32, isOutput=True)

    Exp = mybir.ActivationFunctionType.Exp

    with tile.TileContext(nc) as tc:
        with (
            tc.tile_pool(name="pers", bufs=1) as pers,
            tc.tile_pool(name="hts", bufs=2) as hts,
            tc.tile_pool(name="osb", bufs=3) as osb,
            tc.tile_pool(name="ps", bufs=1, space="PSUM") as ps,
        ):
            # ---- const loads: two packed DMAs, first on the sync queue -----
            cb_sb = pers.tile([128, CW], BF, tag="cb")
            nc.sync.dma_start(cb_sb[:], cb_d[:])
            cf_sb = pers.tile([128, 2], F32, tag="cf")
            nc.sync.dma_start(cf_sb[:], cf_d[:])
            wq_sb = cb_sb[:, 0:256]
            wkv_sb = cb_sb[:, 256:768]
            frhs_sb = cb_sb[0:DK + 1, 768:1280]
            maska_sb = cb_sb[:, 1280:1536]
            maskb_sb = cb_sb[:, 1536:1792]
            id_sb = cb_sb[:, 1792:1920]
            bq_sb = cf_sb[0:DK, 0:1]
            bkv_sb = cf_sb[:, 1:2]

            # ---- input loads, all issued up-front (gpsimd SWDGE) -----------
            # half-group h tile: [128, 1024] bf16, (p, 512*t + d) =
            # X[256h + 2p + t, d]  -> 4KB-contiguous f32 reads per partition.
            knat = [pers.tile([128, 1024], BF, tag=f"knat{h}", name=f"knat{h}")
                    for h in range(NHG)]
            qnat = [pers.tile([128, 1024], BF, tag=f"qnat{h}", name=f"qnat{h}")
                    for h in range(NHG)]
            # k on the scalar HWDGE queue, q on the sync HWDGE queue: two
            # independent DMA queues, ~600ns issue each, parallel engines.
            for h in range(NHG):
                nc.scalar.dma_start(
                    knat[h][:].rearrange("p (two d) -> p two d", two=2),
                    k_d[256 * h:256 * (h + 1), :].rearrange(
                        "(p two) d -> p two d", p=128))
                nc.sync.dma_start(
                    qnat[h][:].rearrange("p (two d) -> p two d", two=2),
                    q_d[256 * h:256 * (h + 1), :].rearrange(
                        "(p two) d -> p two d", p=128))

            # ---- persistent activations -----------------------------------
            # xT layout: col 1024h + 512t + 128c + i  (c = D-chunk, i =
            # intra-class token index, classes C = 2h+t).
            xqT = pers.tile([128, 8192], BF, tag="xqT")
            xkT = pers.tile([128, 8192], BF, tag="xkT")
            qT = pers.tile([DK, S], BF, tag="qT")
            kvT = pers.tile([128, S], BF, tag="kvT")
            # v' for all 16 classes: class C at cols [65C, 65C+64], ones col
            # at 65C+64 (one big memset; evictions overwrite the value cols).
            vpr = pers.tile([128, 65 * NCLS], BF, tag="vpr")
            nc.gpsimd.memset(vpr[:], 1.0)

            # pT storage per class: W[C] = 2048 - 256*(C>>1)
            Ws = [(S - 256 * (C >> 1)) if masked else S for C in range(NCLS)]
            pts = [pers.tile([128, Ws[C]], BF, tag=f"pt{C}", name=f"pt{C}")
                   for C in range(NCLS)]

            # ---- PE warm-up: opens the HAM clock gate while loads fly ------
            wu = pers.tile([128, 512], BF, tag="wu")
            nc.vector.memset(wu[:], 0.0)
            wu_ps = ps.tile([128, 512], F32, tag="sc", name="wu_ps", bufs=2)
            for i in range(10):
                nc.tensor.matmul(wu_ps[:], lhsT=wu[:, 0:128], rhs=wu[:],
                                 start=(i == 0), stop=(i == 9))
            wu2 = pers.tile([128, 1], F32, tag="wu2")
            nc.vector.tensor_copy(wu2[:], wu_ps[:, 0:1])
            nc.gpsimd.dma_start(warm_d[:], wu2[:])

            # ---- per-half-group setup -------------------------------------
            # proj rhs view: [p, t, i] for fixed (h, c)
            xqT_p = xqT[:].rearrange("p (h t c i) -> p h t c i",
                                     h=NHG, t=2, c=4, i=128)
            xkT_p = xkT[:].rearrange("p (h t c i) -> p h t c i",
                                     h=NHG, t=2, c=4, i=128)

            def setup_hg(h):
                sl = slice(256 * h, 256 * (h + 1))
                xsl = slice(1024 * h, 1024 * (h + 1))
                # PE transposes into one bf16 psum bank per tensor, single
                # DVE eviction each
                for nat, xT, who in ((knat[h], xkT, "k"), (qnat[h], xqT, "q")):
                    tr = ps.tile([128, 1024], BF, tag="tr",
                                 name=f"tr_{who}{h}", bufs=2)
                    for t in range(2):
                        for c in range(4):
                            o = 512 * t + 128 * c
                            nc.tensor.transpose(tr[:, o:o + 128],
                                                nat[:, o:o + 128], id_sb)
                    nc.vector.tensor_copy(xT[:, xsl], tr[:])
                # kv projection
                pkv = ps.tile([128, 256], F32, tag="pj", name=f"pkv{h}", bufs=1)
                for cc in range(4):
                    nc.tensor.matmul(pkv[:],
                                     lhsT=wkv_sb[:, cc * 128:(cc + 1) * 128],
                                     rhs=xkT_p[:, h, :, cc, :],
                                     start=(cc == 0), stop=(cc == 3))
                nc.vector.tensor_scalar_add(kvT[:, sl], pkv[:], bkv_sb)
                # q projection
                pq = ps.tile([DK, 256], F32, tag="pj", name=f"pq{h}", bufs=1)
                for cc in range(4):
                    nc.tensor.matmul(pq[:],
                                     lhsT=wq_sb[:, cc * DK:(cc + 1) * DK],
                                     rhs=xqT_p[:, h, :, cc, :],
                                     start=(cc == 0), stop=(cc == 3))
                nc.vector.tensor_scalar_add(qT[:, sl], pq[:], bq_sb)
                # v' for the two classes of this half-group (one psum, one
                # strided eviction)
                pv = ps.tile([128, 128], BF, tag="pv", name=f"pv{h}", bufs=1)
                for t in range(2):
                    C = 2 * h + t
                    nc.tensor.transpose(pv[:, 64 * t:64 * t + 64],
                                        kvT[64:128, 128 * C:128 * (C + 1)],
                                        id_sb[64:128, 64:128])
                nc.vector.tensor_copy(
                    vpr[:].rearrange("p (C x) -> p C x", x=65)
                        [:, 2 * h:2 * h + 2, 0:64],
                    pv[:].rearrange("p (C x) -> p C x", x=64))

            # ---- per-piece attention sweep --------------------------------
            out_v = out_d[:].rearrange("(h p two) d -> h two p d", h=NHG, p=128)

            def piece(p):
                Cmax = 4 * p + 3 if masked else NCLS - 1
                # scores + exp + mask
                for C in range(Cmax + 1):
                    h2 = C >> 1
                    if masked and h2 == 2 * p + 1:
                        local0, w = 256, 256
                    else:
                        local0, w = 0, 512
                    g0 = 512 * p + local0
                    psc = ps.tile([128, 512], F32, tag="sc", name=f"sc_{C}_{p}",
                                  bufs=2)
                    nc.tensor.matmul(psc[:, 0:w],
                                     lhsT=kvT[0:DK, 128 * C:128 * (C + 1)],
                                     rhs=qT[:, g0:g0 + w],
                                     start=True, stop=True,
                                     skip_group_check=True)
                    if masked:
                        x = g0 - 512 * (h2 >> 1) - 256 * (h2 & 1)
                    else:
                        x = 512 * p
                    nc.scalar.activation(pts[C][:, x:x + w], psc[:, 0:w],
                                         Exp, scale=0.125)
                    if masked and (h2 == 2 * p or h2 == 2 * p + 1):
                        msk = maska_sb if (C & 1) == 0 else maskb_sb
                        nc.gpsimd.tensor_tensor(pts[C][:, 0:256],
                                                pts[C][:, 0:256], msk,
                                                mybir.AluOpType.mult)
                # head accumulation
                hacc = ps.tile([DK + 1, 512], F32, tag="ha", name=f"ha{p}",
                               bufs=1)
                for C in range(Cmax + 1):
                    h2 = C >> 1
                    if masked and h2 == 2 * p + 1:
                        lo, w = 256, 256
                    else:
                        lo, w = 0, 512
                    if masked:
                        x = 512 * p + lo - 512 * (h2 >> 1) - 256 * (h2 & 1)
                    else:
                        x = 512 * p
                    nc.tensor.matmul(hacc[:, lo:lo + w],
                                     lhsT=vpr[:, 65 * C:65 * (C + 1)],
                                     rhs=pts[C][:, x:x + w],
                                     start=(C == 0), stop=(C == Cmax),
                                     skip_group_check=True)
                ht4 = hts.tile([DK + 1, 512], BF, tag="ht", name=f"ht{p}")
                nc.vector.tensor_copy(ht4[:], hacc[:])
                # finalize the 4 classes of this piece; l-row -> columns via
                # one PE transpose + one batched reciprocal
                # bf16 transpose outs must land 4-byte aligned: stride-2 cols
                pl = ps.tile([128, 8], BF, tag="pv", name=f"pl{p}", bufs=1)
                for b in range(4):
                    nc.tensor.transpose(pl[:, 2 * b:2 * b + 1],
                                        ht4[DK:DK + 1, 128 * b:128 * (b + 1)],
                                        id_sb[64:65, 64:65])
                r = hts.tile([128, 4], F32, tag="r", name=f"r{p}")
                nc.vector.reciprocal(
                    r[:], pl[:].rearrange("p (b two) -> p b two", two=2)[:, :, 0])
                for b in range(4):
                    C = 4 * p + b
                    po = ps.tile([128, 512], F32, tag="po", name=f"po{C}",
                                 bufs=1)
                    nc.tensor.matmul(po[:], lhsT=ht4[:, 128 * b:128 * (b + 1)],
                                     rhs=frhs_sb, start=True, stop=True)
                    ot = osb.tile([128, D], BF, tag="ot", name=f"ot{C}")
                    nc.scalar.mul(ot[:], po[:], r[:, b:b + 1])
                    nc.gpsimd.dma_start(out_v[C >> 1, C & 1], ot[:])

            for h in range(NHG):
                setup_hg(h)
                if h % 2 == 1:
                    piece(h // 2)

    _split_sync_waits(nc)
    return nc


_NC_CACHE = {}


def _get_nc(masked: bool):
    if masked not in _NC_CACHE:
        _NC_CACHE[masked] = _build_nc(masked)
    return _NC_CACHE[masked]


# ---------------------------------------------------------------------------
def kernel(query, key, value, Wq, bq, Wk, bk, Wv, bv, Wo, bo, training):
    query = np.asarray(query, dtype=np.float32)
    key = np.asarray(key, dtype=np.float32)
    Wq = np.asarray(Wq, dtype=np.float64)
    Wk = np.asarray(Wk, dtype=np.float64)
    Wv = np.asarray(Wv, dtype=np.float64)
    Wo = np.asarray(Wo, dtype=np.float64)
    bq_h = np.asarray(bq, dtype=np.float32).reshape(DK, 1)
    bk_h = np.asarray(bk, dtype=np.float32).reshape(DK, 1)
    bv_h = np.asarray(bv, dtype=np.float32).reshape(DK, 1)
    bo_h = np.asarray(bo, dtype=np.float64)
    masked = bool(np.asarray(training).item())

    B = query.shape[0]
    query_bf = query.astype(ml_dtypes.bfloat16)
    key_bf = key.astype(ml_dtypes.bfloat16)
    wq_h = Wq.astype(ml_dtypes.bfloat16)
    wkv_h = np.concatenate([Wk, Wv], axis=1).astype(ml_dtypes.bfloat16)
    bkv_h = np.concatenate([bk_h, bv_h], axis=0)
    wo_eff = Wo.reshape(H, DK, D).sum(axis=0)
    frhs_h = np.concatenate([wo_eff, bo_h[None, :]], axis=0).astype(ml_dtypes.bfloat16)
    jj, ii = np.meshgrid(np.arange(128), np.arange(128), indexing="ij")
    t1v = (jj <= ii).astype(np.float64)   # t2<=t1: key j visible iff j<=i
    t0v = (jj < ii).astype(np.float64)    # t2>t1:  key j visible iff j<i
    maska_h = np.concatenate([t1v, t1v], axis=1).astype(ml_dtypes.bfloat16)
    maskb_h = np.concatenate([t0v, t1v], axis=1).astype(ml_dtypes.bfloat16)
    id_h = np.eye(128, dtype=ml_dtypes.bfloat16)

    # packed const blobs (see _build_nc layout)
    wq_p = np.zeros((128, 256), ml_dtypes.bfloat16)
    for cc in range(4):
        wq_p[:, cc * 64:(cc + 1) * 64] = wq_h[cc * 128:(cc + 1) * 128, :]
    wkv_p = np.zeros((128, 512), ml_dtypes.bfloat16)
    for cc in range(4):
        wkv_p[:, cc * 128:(cc + 1) * 128] = wkv_h[cc * 128:(cc + 1) * 128, :]
    frhs_p = np.zeros((128, 512), ml_dtypes.bfloat16)
    frhs_p[0:DK + 1, :] = frhs_h
    cb = np.concatenate([wq_p, wkv_p, frhs_p, maska_h, maskb_h, id_h], axis=1)
    cf = np.zeros((128, 2), np.float32)
    cf[0:DK, 0] = bq_h[:, 0]
    cf[:, 1] = bkv_h[:, 0]
    consts = {"cb": np.ascontiguousarray(cb), "cf": cf}
    in_maps = [dict(consts, query=np.ascontiguousarray(query_bf[i]),
                    key=np.ascontiguousarray(key_bf[i])) for i in range(B)]

    global _last_in_maps
    _last_in_maps = in_maps
    nc = _get_nc(masked)
    res = run_bass_kernel_spmd(nc, in_maps, core_ids=list(range(B)))
    return np.stack([np.asarray(res.results[i]["out"], dtype=np.float32)
                     for i in range(B)])
